# revision 1
# baseline (speedup 1.0000x reference)
"""Trainium2 Bass kernel for a 6-layer transformer encoder (v3).

nn_Encoder: B=8, S=2048, D=512, NHEAD=8, D_FF=2048.

Strategy
--------
Data-parallel: one batch element per NeuronCore, no collectives.
The reference's reshape-without-transpose makes attention block-diagonal
over 8 slabs of 256 tokens; each slab's (256 x 512) block self-attends
as a (2048 x 64) matrix. Rows are permuted to (j*256 + s_local) order so
every operand is a natural slice of transposed activations.

All matmuls run in bf16 (weights converted host-side): on HW, fp32r
weight loads bypass Fast-Weight-Load and stream at ~0.7 ns/col, while
bf16 hits ~0.4 ns/col. Attention-score error is softmax-averaged and
residual-path error stays ~1.5e-2 max-rel (gate 2e-2).

Schedule: one flat software pipeline over (layer, slab-pair) stages.
Attention's exp() stream makes its inner loop ACT-bound, so the PE is
fed with FFN matmuls of the previous pair and projections of the next
pair (cross-layer at boundaries) popped from a steal queue. PSUM
discipline: consecutive matmuls never hit the same bank; accumulation
chains are emitted as interleaved pairs.
"""

import numpy as np

P = 128
D = 512
S = 2048
FF = 2048
NH = 8          # slabs
T = 256         # tokens per slab
DH = 64
G = D // P      # 4
GF = FF // P    # 16
B = 8
EPS = 1e-5
N_LAYERS = 6
NPAIR = 4       # slab pairs
PT = 2 * T      # tokens per pair (512)

_CACHE = {}


def _build(n_layers=N_LAYERS, rep=1, abl=""):
    from collections import deque
    import concourse.bass as bass
    import concourse.tile as tile
    from concourse import bacc, mybir

    CDT = mybir.dt.bfloat16
    F32 = mybir.dt.float32
    F32R = mybir.dt.float32r
    U32 = mybir.dt.uint32
    AF = mybir.ActivationFunctionType
    OP = mybir.AluOpType

    nc = bacc.Bacc("TRN2", target_bir_lowering=False)

    xT_d = nc.dram_tensor("xT", (D, S), CDT, kind="ExternalInput")
    wq_d = nc.dram_tensor("wq", (N_LAYERS, D, D), CDT, kind="ExternalInput")
    wk_d = nc.dram_tensor("wk", (N_LAYERS, D, D), CDT, kind="ExternalInput")
    wv_d = nc.dram_tensor("wv", (N_LAYERS, D, D), CDT, kind="ExternalInput")
    wo_d = nc.dram_tensor("wo", (N_LAYERS, D, D), CDT, kind="ExternalInput")
    w1_d = nc.dram_tensor("w1", (N_LAYERS, D, FF), CDT, kind="ExternalInput")
    w2_d = nc.dram_tensor("w2", (N_LAYERS, FF, D), CDT, kind="ExternalInput")
    b1_d = nc.dram_tensor("b1", (N_LAYERS, FF), F32, kind="ExternalInput")
    b2_d = nc.dram_tensor("b2", (N_LAYERS, D), F32, kind="ExternalInput")
    g1_d = nc.dram_tensor("g1", (N_LAYERS, D), F32, kind="ExternalInput")
    bt1_d = nc.dram_tensor("beta1", (N_LAYERS, D), F32, kind="ExternalInput")
    g2_d = nc.dram_tensor("g2", (N_LAYERS, D), F32, kind="ExternalInput")
    bt2_d = nc.dram_tensor("beta2", (N_LAYERS, D), F32, kind="ExternalInput")
    out_d = nc.dram_tensor("out", (D, S), CDT, kind="ExternalOutput")

    QO, KO, VO = 0, D, 2 * D
    total = n_layers * rep

    with tile.TileContext(nc) as tc:
        with tc.tile_pool(name="const", bufs=1) as cpool, \
             tc.tile_pool(name="x", bufs=1) as xpool, \
             tc.tile_pool(name="wqkv", bufs=2) as wqkvpool, \
             tc.tile_pool(name="wos", bufs=1) as wospool, \
             tc.tile_pool(name="w1", bufs=1) as w1pool, \
             tc.tile_pool(name="w2", bufs=1) as w2pool, \
             tc.tile_pool(name="par", bufs=2) as parpool, \
             tc.tile_pool(name="qkv", bufs=2) as qkvpool, \
             tc.tile_pool(name="oT", bufs=2) as oTpool, \
             tc.tile_pool(name="fT", bufs=2) as fTpool, \
             tc.tile_pool(name="r", bufs=2) as rpool, \
             tc.tile_pool(name="pX", bufs=2) as pXpool, \
             tc.tile_pool(name="sm", bufs=2) as small, \
             tc.tile_pool(name="um", bufs=2, space="PSUM") as umpool, \
             tc.tile_pool(name="po", bufs=(1 if abl == "expw" else 2),
                          space="PSUM") as popool, \
             tc.tile_pool(name="pp", bufs=2, space="PSUM") as pppool:

            # ---------------- constants ----------------
            ones_f = cpool.tile([P, 16], F32, tag="ones_f")
            nc.vector.memset(ones_f, 1.0)
            ones_c = cpool.tile([P, 16], CDT, tag="ones_c")
            nc.vector.tensor_copy(out=ones_c, in_=ones_f)
            ones1 = cpool.tile([P, 1], CDT, tag="ones1")      # LN-stats lhsT
            nc.vector.tensor_copy(out=ones1, in_=ones_f[:, 0:1])
            ones1r = cpool.tile([P, 1], F32R, tag="ones1r")
            nc.vector.tensor_copy(out=ones1r, in_=ones_f[:, 0:1])

            # persistent activations, transposed: xT_s[p, g, s] = x[s, g*128+p]
            xT_s = xpool.tile([P, G, S], CDT, tag="xT")
            for g in range(G):
                nc.sync.dma_start(xT_s[:, g, :], xT_d[g * P:(g + 1) * P, :])

            def quake_rsqrt(s1, s2, s3):
                """s3 holds var+eps on entry; writes rsqrt(s3) into s1.
                All [1, PT] f32 slices. Quake-III seed + 2 Newton steps,
                entirely on DVE (keeps ACT exp/relu-only)."""
                nc.vector.tensor_copy(out=s2, in_=s3.bitcast(U32))
                nc.vector.tensor_scalar(out=s2, in0=s2,
                                        scalar1=-0.5, scalar2=float(0x5F3759DF),
                                        op0=OP.mult, op1=OP.add)
                nc.vector.tensor_copy(out=s2.bitcast(U32), in_=s2)
                nc.vector.tensor_tensor(s1, s2, s2, OP.mult)
                nc.vector.tensor_tensor(s1, s3, s1, OP.mult)
                nc.vector.tensor_scalar(out=s1, in0=s1, scalar1=-0.5, scalar2=1.5,
                                        op0=OP.mult, op1=OP.add)
                nc.vector.tensor_tensor(s2, s2, s1, OP.mult)
                nc.vector.tensor_tensor(s1, s2, s2, OP.mult)
                nc.vector.tensor_tensor(s1, s3, s1, OP.mult)
                nc.vector.tensor_scalar(out=s1, in0=s1, scalar1=-0.5, scalar2=1.5,
                                        op0=OP.mult, op1=OP.add)
                nc.vector.tensor_tensor(s1, s2, s1, OP.mult)

            W = {}       # li -> layer weight tiles
            PAIR = {}    # (li, p) -> projection tiles; (li, p, "oT"/"fT")

            def setup(li):
                l = li % n_layers
                wqkv = wqkvpool.tile([P, G, 3 * D], CDT, tag="wqkv",
                                     name=f"wqkv_{li}")
                for wi, w_d in enumerate((wq_d, wk_d, wv_d)):
                    for ki in range(G):
                        nc.sync.dma_start(wqkv[:, ki, wi * D:(wi + 1) * D],
                                          w_d[l, ki * P:(ki + 1) * P, :])
                wos = wospool.tile([P, G, D], CDT, tag="wos", name=f"wos_{li}")
                for ki in range(G):
                    nc.sync.dma_start(wos[:, ki, :], wo_d[l, ki * P:(ki + 1) * P, :])
                w1s = w1pool.tile([P, G, FF], CDT, tag="w1s", name=f"w1s_{li}")
                for ki in range(G):
                    nc.sync.dma_start(w1s[:, ki, :], w1_d[l, ki * P:(ki + 1) * P, :])
                w2s = w2pool.tile([P, GF, D], CDT, tag="w2s", name=f"w2s_{li}")
                for fi in range(GF):
                    nc.sync.dma_start(w2s[:, fi, :], w2_d[l, fi * P:(fi + 1) * P, :])
                g1s = parpool.tile([P, G], F32, tag="g1", name=f"g1_{li}")
                bt1s = parpool.tile([P, G], F32, tag="bt1", name=f"bt1_{li}")
                g2s = parpool.tile([P, G], F32, tag="g2", name=f"g2_{li}")
                bt2s = parpool.tile([P, G], F32, tag="bt2", name=f"bt2_{li}")
                b2s = parpool.tile([P, G], F32, tag="b2", name=f"b2_{li}")
                for t_, d_ in ((g1s, g1_d), (bt1s, bt1_d), (g2s, g2_d),
                               (bt2s, bt2_d), (b2s, b2_d)):
                    nc.sync.dma_start(t_, d_[l].rearrange("(o p) -> p o", p=P))
                b1c = parpool.tile([P, GF], F32, tag="b1c", name=f"b1c_{li}")
                nc.sync.dma_start(b1c, b1_d[l].rearrange("(o p) -> p o", p=P))
                W[li] = dict(wqkv=wqkv, wos=wos, w1s=w1s, w2s=w2s, g1s=g1s,
                             bt1s=bt1s, g2s=g2s, bt2s=bt2s, b2s=b2s, b1c=b1c)

            def layernorm_pair(li, p, r, r2, gam, bet, uid):
                """LN over features of r [P, G, PT]; writes xT_s[:, :, pair]."""
                ts = slice(p * PT, (p + 1) * PT)
                nc.vector.tensor_tensor(r2[:], r[:], r[:], OP.mult)
                stA = pppool.tile([1, PT], F32, tag="pp", name=f"stA_{li}_{p}_{uid}")
                stB = pppool.tile([1, PT], F32, tag="pp", name=f"stB_{li}_{p}_{uid}")
                for gi in range(G):
                    nc.tensor.matmul(stA, ones1r, r[:, gi, :],
                                     start=(gi == 0), stop=(gi == G - 1))
                    nc.tensor.matmul(stB, ones1, r2[:, gi, :],
                                     start=(gi == 0), stop=(gi == G - 1))
                ms = small.tile([1, 4, PT], F32, tag="lnsc", name=f"ms_{li}_{p}_{uid}")
                nc.vector.tensor_scalar_mul(ms[0:1, 0, :], stA, 1.0 / D)
                nc.vector.tensor_scalar_mul(ms[0:1, 3, :], stB, 1.0 / D)
                nc.vector.tensor_tensor(ms[0:1, 2, :], ms[0:1, 0, :],
                                        ms[0:1, 0, :], OP.mult)
                nc.vector.tensor_tensor(ms[0:1, 2, :], ms[0:1, 3, :],
                                        ms[0:1, 2, :], OP.subtract)
                nc.vector.tensor_scalar(out=ms[0:1, 3, :], in0=ms[0:1, 2, :],
                                        scalar1=1.0, scalar2=EPS,
                                        op0=OP.mult, op1=OP.add)
                quake_rsqrt(ms[0:1, 1, :], ms[0:1, 2, :], ms[0:1, 3, :])
                bc = small.tile([P, 2, PT], F32, tag="lnsc", name=f"bc_{li}_{p}_{uid}")
                nc.gpsimd.partition_broadcast(bc, ms[0:1, 0:2, :])
                mu_b = bc[:, 0:1, :].to_broadcast((P, G, PT))
                sd_b = bc[:, 1:2, :].to_broadcast((P, G, PT))
                nc.vector.tensor_tensor(r[:], r[:], mu_b, OP.subtract)
                nc.vector.tensor_tensor(r[:], r[:], sd_b, OP.mult)
                gam_b = gam[:, :, None].to_broadcast((P, G, PT))
                bet_b = bet[:, :, None].to_broadcast((P, G, PT))
                nc.vector.tensor_tensor(r[:], r[:], gam_b, OP.mult)
                nc.vector.tensor_tensor(xT_s[:, :, ts], r[:], bet_b, OP.add)

            def proj_pieces(li, p):
                """q/k/v projections for (layer li, pair p)."""
                w = W[li]
                ts = slice(p * PT, (p + 1) * PT)
                qT = qkvpool.tile([P, G, PT], CDT, tag="qT", name=f"qT_{li}_{p}")
                kT = qkvpool.tile([P, G, PT], CDT, tag="kT", name=f"kT_{li}_{p}")
                ksx = qkvpool.tile([P, G, PT], CDT, tag="ksx", name=f"ksx_{li}_{p}")
                v65a = qkvpool.tile([P, 2, 8, 65], CDT, tag="v65a",
                                    name=f"v65_{li}_{2 * p}")
                v65b = qkvpool.tile([P, 2, 8, 65], CDT, tag="v65b",
                                    name=f"v65_{li}_{2 * p + 1}")
                PAIR[(li, p)] = (qT, kT, ksx, v65a, v65b)
                pieces = []

                def ones_col():
                    nc.vector.tensor_copy(out=v65a[:, :, :, 64:65], in_=ones_c)
                    nc.vector.tensor_copy(out=v65b[:, :, :, 64:65], in_=ones_c)
                pieces.append(ones_col)

                def qk_mm(pq, pk, g, ki0):
                    for ki in (ki0, ki0 + 1):
                        nc.tensor.matmul(pq, w["wqkv"][:, ki, QO + g * P:QO + (g + 1) * P],
                                         xT_s[:, ki, ts],
                                         start=(ki == 0), stop=(ki == G - 1))
                        nc.tensor.matmul(pk, w["wqkv"][:, ki, KO + g * P:KO + (g + 1) * P],
                                         xT_s[:, ki, ts],
                                         start=(ki == 0), stop=(ki == G - 1))

                def qk_copy(pq, pk, g):
                    nc.vector.tensor_copy(out=qT[:, g, :], in_=pq)
                    nc.vector.tensor_copy(out=kT[:, g, :], in_=pk)
                    nc.sync.dma_start(ksx[0:64, g, :], kT[64:128, g, :])
                    nc.sync.dma_start(ksx[64:128, g, :], kT[0:64, g, :])

                for g in range(G):
                    pq = pppool.tile([P, PT], F32, tag="pp", name=f"pq_{li}_{p}_{g}")
                    pk = pppool.tile([P, PT], F32, tag="pp", name=f"pk_{li}_{p}_{g}")
                    for ki0 in (0, 2):
                        pieces.append(lambda pq=pq, pk=pk, g=g, ki0=ki0:
                                      qk_mm(pq, pk, g, ki0))
                    pieces.append(lambda pq=pq, pk=pk, g=g: qk_copy(pq, pk, g))

                def v_mm(pv1, pv2, h, ki0):
                    hs = h * T
                    for ki in (ki0, ki0 + 1):
                        nc.tensor.matmul(pv1, xT_s[:, ki, hs:hs + P],
                                         w["wqkv"][:, ki, VO:VO + D],
                                         start=(ki == 0), stop=(ki == G - 1))
                        nc.tensor.matmul(pv2, xT_s[:, ki, hs + P:hs + 2 * P],
                                         w["wqkv"][:, ki, VO:VO + D],
                                         start=(ki == 0), stop=(ki == G - 1))

                def v_copy(pv1, pv2, v65):
                    nc.vector.tensor_copy(out=v65[:, 0, :, 0:64], in_=pv1)
                    nc.vector.tensor_copy(out=v65[:, 1, :, 0:64], in_=pv2)

                for ci, v65 in ((0, v65a), (1, v65b)):
                    h = 2 * p + ci
                    pv1 = pppool.tile([P, D], F32, tag="pp", name=f"pv1_{li}_{h}")
                    pv2 = pppool.tile([P, D], F32, tag="pp", name=f"pv2_{li}_{h}")
                    for ki0 in (0, 2):
                        pieces.append(lambda pv1=pv1, pv2=pv2, h=h, ki0=ki0:
                                      v_mm(pv1, pv2, h, ki0))
                    pieces.append(lambda pv1=pv1, pv2=pv2, v65=v65:
                                  v_copy(pv1, pv2, v65))
                return pieces

            def attn(li, h, steal):
                """Attention for slab h; pops steal pieces to fill the PE."""
                p, c = h // 2, h % 2
                qT, kT, ksx, v65a, v65b = PAIR[(li, p)]
                v65 = v65a if c == 0 else v65b
                if c == 0:
                    oT = oTpool.tile([P, G, PT], CDT, tag="oT", name=f"oT_{li}_{p}")
                    PAIR[(li, p, "oT")] = oT
                else:
                    oT = PAIR[(li, p, "oT")]
                cs = slice(c * T, (c + 1) * T)

                if abl in ("noattn",):
                    if c == 0:
                        nc.vector.memset(oT, 0.01)
                    for _ in range(32):
                        if steal:
                            steal.popleft()()
                    return
                if abl == "expw":
                    for sh in range(2):
                        poE = popool.tile([65, 512], F32, tag="poE",
                                          name=f"poE_{li}_{h}_{sh}")
                        poO = popool.tile([65, 512], F32, tag="poO",
                                          name=f"poO_{li}_{h}_{sh}")
                        rhsE = qT[0:64, 2 * sh:2 * sh + 2, cs]
                        rhsO = qT[64:128, 2 * sh:2 * sh + 2, cs]
                        hist = []
                        for t in range(16):
                            jb, cc = t // 2, t % 2
                            um = umpool.tile([P, 1024], F32, tag="um",
                                             name=f"um_{li}_{h}_{sh}_{t}")
                            kcol = slice(c * T + cc * P, c * T + (cc + 1) * P)
                            if jb % 2 == 0:
                                lhsE = kT[0:64, jb // 2, kcol]
                                lhsO = ksx[64:128, jb // 2, kcol]
                            else:
                                lhsE = ksx[0:64, jb // 2, kcol]
                                lhsO = kT[64:128, jb // 2, kcol]
                            nc.tensor.matmul(um[:, 0:512], lhsE, rhsE,
                                             start=True, stop=True)
                            nc.tensor.matmul(um[:, 512:1024], lhsO, rhsO,
                                             start=True, stop=True)
                            pX = pXpool.tile([P, 1024], CDT, tag="pXE",
                                             name=f"pX_{li}_{h}_{sh}_{t}")
                            nc.scalar.activation(out=pX, in_=um, func=AF.Exp,
                                                 scale=0.125)
                            hist.append((pX, t))
                            if len(hist) > 2:   # AV lags 2 t-chunks
                                pXp, tp = hist.pop(0)
                                jbp, ccp = tp // 2, tp % 2
                                nc.tensor.matmul(poE, v65[:, ccp, jbp, :],
                                                 pXp[:, 0:512],
                                                 start=(tp == 0), stop=False)
                                nc.tensor.matmul(poO, v65[:, ccp, jbp, :],
                                                 pXp[:, 512:1024],
                                                 start=(tp == 0), stop=False)
                            if steal:
                                steal.popleft()()
                        for pXp, tp in hist:
                            jbp, ccp = tp // 2, tp % 2
                            nc.tensor.matmul(poE, v65[:, ccp, jbp, :],
                                             pXp[:, 0:512],
                                             start=(tp == 0), stop=(tp == 15))
                            nc.tensor.matmul(poO, v65[:, ccp, jbp, :],
                                             pXp[:, 512:1024],
                                             start=(tp == 0), stop=(tp == 15))
                        rec = small.tile([1, 1024], F32, tag="den",
                                         name=f"rec_{li}_{h}_{sh}")
                        nc.vector.reciprocal(out=rec[:, 0:512], in_=poE[64:65, :])
                        nc.vector.reciprocal(out=rec[:, 512:1024],
                                             in_=poO[64:65, :])
                        bcd = small.tile([64, 1024], F32, tag="den",
                                         name=f"bcd_{li}_{h}_{sh}")
                        nc.gpsimd.partition_broadcast(bcd, rec)
                        nc.vector.tensor_tensor(
                            oT[0:64, 2 * sh:2 * sh + 2, cs], poE[0:64, :],
                            bcd[0:64, 0:512], OP.mult)
                        nc.vector.tensor_tensor(
                            oT[64:128, 2 * sh:2 * sh + 2, cs], poO[0:64, :],
                            bcd[0:64, 512:1024], OP.mult)
                    return
                for sh in range(2):  # m-halves {0,1}, {2,3}
                    poE = popool.tile([65, 512], F32, tag="poE",
                                      name=f"poE_{li}_{h}_{sh}")
                    poO = popool.tile([65, 512], F32, tag="poO",
                                      name=f"poO_{li}_{h}_{sh}")
                    rhsE = qT[0:64, 2 * sh:2 * sh + 2, cs]
                    rhsO = qT[64:128, 2 * sh:2 * sh + 2, cs]
                    prev = None
                    for t in range(16):  # t-chunk = (jb, cc)
                        jb, cc = t // 2, t % 2
                        umE = umpool.tile([P, 512], F32, tag="um",
                                          name=f"umE_{li}_{h}_{sh}_{t}")
                        umO = umpool.tile([P, 512], F32, tag="um",
                                          name=f"umO_{li}_{h}_{sh}_{t}")
                        kcol = slice(c * T + cc * P, c * T + (cc + 1) * P)
                        if jb % 2 == 0:
                            lhsE = kT[0:64, jb // 2, kcol]
                            lhsO = ksx[64:128, jb // 2, kcol]
                        else:
                            lhsE = ksx[0:64, jb // 2, kcol]
                            lhsO = kT[64:128, jb // 2, kcol]
                        nc.tensor.matmul(umE, lhsE, rhsE, start=True, stop=True)
                        nc.tensor.matmul(umO, lhsO, rhsO, start=True, stop=True)
                        if abl == "noexp" and t > 0:
                            pXE, pXO = prev[0], prev[1]
                        else:
                            pXE = pXpool.tile([P, 512], CDT, tag="pXE",
                                              name=f"pXE_{li}_{h}_{sh}_{t}")
                            pXO = pXpool.tile([P, 512], CDT, tag="pXO",
                                              name=f"pXO_{li}_{h}_{sh}_{t}")
                        if abl != "noexp":
                            nc.scalar.activation(out=pXE, in_=umE, func=AF.Exp,
                                                 scale=0.125)
                            nc.scalar.activation(out=pXO, in_=umO, func=AF.Exp,
                                                 scale=0.125)
                        elif t == 0:
                            nc.vector.memset(pXE, 0.01)
                            nc.vector.memset(pXO, 0.01)
                        if abl in ("noav", "noexp"):
                            prev = (pXE, pXO, t)
                            if steal:
                                steal.popleft()()
                            continue
                        if prev is not None:
                            pEp, pOp, tp = prev
                            jbp, ccp = tp // 2, tp % 2
                            nc.tensor.matmul(poE, v65[:, ccp, jbp, :], pEp,
                                             start=(tp == 0), stop=False)
                            nc.tensor.matmul(poO, v65[:, ccp, jbp, :], pOp,
                                             start=(tp == 0), stop=False)
                        prev = (pXE, pXO, t)
                        if steal:
                            steal.popleft()()
                    if abl in ("noav", "noexp"):
                        nc.tensor.matmul(poE, v65[:, 0, 0, :], pXE,
                                         start=True, stop=True)
                        nc.tensor.matmul(poO, v65[:, 0, 0, :], pXO,
                                         start=True, stop=True)
                    else:
                        pEp, pOp, tp = prev
                        jbp, ccp = tp // 2, tp % 2
                        nc.tensor.matmul(poE, v65[:, ccp, jbp, :], pEp,
                                         start=False, stop=True)
                        nc.tensor.matmul(poO, v65[:, ccp, jbp, :], pOp,
                                         start=False, stop=True)

                    # denominators -> oT (normalized, feature-major)
                    rec = small.tile([1, 1024], F32, tag="den",
                                     name=f"rec_{li}_{h}_{sh}")
                    nc.vector.reciprocal(out=rec[:, 0:512], in_=poE[64:65, :])
                    nc.vector.reciprocal(out=rec[:, 512:1024], in_=poO[64:65, :])
                    bcd = small.tile([64, 1024], F32, tag="den",
                                     name=f"bcd_{li}_{h}_{sh}")
                    nc.gpsimd.partition_broadcast(bcd, rec)
                    nc.vector.tensor_tensor(
                        oT[0:64, 2 * sh:2 * sh + 2, cs], poE[0:64, :],
                        bcd[0:64, 0:512], OP.mult)
                    nc.vector.tensor_tensor(
                        oT[64:128, 2 * sh:2 * sh + 2, cs], poO[0:64, :],
                        bcd[0:64, 512:1024], OP.mult)

            def wo_ln1(li, p, steal):
                """wo projection + residual + LN1 for pair p."""
                w = W[li]
                ts = slice(p * PT, (p + 1) * PT)
                oT = PAIR[(li, p, "oT")]
                r = rpool.tile([P, G, PT], F32R, tag="r", name=f"r1_{li}_{p}")
                r2 = rpool.tile([P, G, PT], CDT, tag="r2", name=f"r12_{li}_{p}")
                for gpair in range(2):
                    pyA = pppool.tile([P, PT], F32, tag="pp",
                                      name=f"wo{li}_{p}_{gpair}a")
                    pyB = pppool.tile([P, PT], F32, tag="pp",
                                      name=f"wo{li}_{p}_{gpair}b")
                    gA, gB = 2 * gpair, 2 * gpair + 1
                    for gi in range(G):
                        nc.tensor.matmul(pyA, w["wos"][:, gi, gA * P:(gA + 1) * P],
                                         oT[:, gi, :],
                                         start=(gi == 0), stop=(gi == G - 1))
                        nc.tensor.matmul(pyB, w["wos"][:, gi, gB * P:(gB + 1) * P],
                                         oT[:, gi, :],
                                         start=(gi == 0), stop=(gi == G - 1))
                        if steal:
                            steal.popleft()()
                    nc.vector.tensor_tensor(r[:, gA, :], pyA, xT_s[:, gA, ts], OP.add)
                    nc.vector.tensor_tensor(r[:, gB, :], pyB, xT_s[:, gB, ts], OP.add)
                layernorm_pair(li, p, r, r2, w["g1s"], w["bt1s"], "a")

            def ffn1_pieces(li, p):
                w = W[li]
                ts = slice(p * PT, (p + 1) * PT)
                fT = fTpool.tile([P, GF, PT], CDT, tag="fT", name=f"fT_{li}_{p}")
                PAIR[(li, p, "fT")] = fT
                pieces = []

                def mm(ppA, ppB, dkA, dkB, ki0):
                    for ki in (ki0, ki0 + 1):
                        nc.tensor.matmul(ppA, w["w1s"][:, ki, dkA * P:(dkA + 1) * P],
                                         xT_s[:, ki, ts],
                                         start=(ki == 0), stop=(ki == G - 1))
                        nc.tensor.matmul(ppB, w["w1s"][:, ki, dkB * P:(dkB + 1) * P],
                                         xT_s[:, ki, ts],
                                         start=(ki == 0), stop=(ki == G - 1))

                def relu(ppA, ppB, dkA, dkB):
                    nc.scalar.activation(out=fT[:, dkA, :], in_=ppA, func=AF.Relu,
                                         bias=w["b1c"][:, dkA:dkA + 1], scale=1.0)
                    nc.scalar.activation(out=fT[:, dkB, :], in_=ppB, func=AF.Relu,
                                         bias=w["b1c"][:, dkB:dkB + 1], scale=1.0)

                for dk in range(0, GF, 2):
                    ppA = pppool.tile([P, PT], F32, tag="pp", name=f"f1a_{li}_{p}_{dk}")
                    ppB = pppool.tile([P, PT], F32, tag="pp", name=f"f1b_{li}_{p}_{dk}")
                    for ki0 in (0, 2):
                        pieces.append(lambda ppA=ppA, ppB=ppB, dk=dk, ki0=ki0:
                                      mm(ppA, ppB, dk, dk + 1, ki0))
                    pieces.append(lambda ppA=ppA, ppB=ppB, dk=dk:
                                  relu(ppA, ppB, dk, dk + 1))
                return pieces

            def ffn2_pieces(li, p):
                w = W[li]
                ts = slice(p * PT, (p + 1) * PT)
                fT = PAIR[(li, p, "fT")]
                r = rpool.tile([P, G, PT], F32R, tag="r", name=f"r2a_{li}_{p}")
                r2 = rpool.tile([P, G, PT], CDT, tag="r2", name=f"r2b_{li}_{p}")
                pieces = []

                def mm(ppA, ppB, gA, gB, fi0):
                    for fi in range(fi0, fi0 + 4):
                        nc.tensor.matmul(ppA, w["w2s"][:, fi, gA * P:(gA + 1) * P],
                                         fT[:, fi, :],
                                         start=(fi == 0), stop=(fi == GF - 1))
                        nc.tensor.matmul(ppB, w["w2s"][:, fi, gB * P:(gB + 1) * P],
                                         fT[:, fi, :],
                                         start=(fi == 0), stop=(fi == GF - 1))

                def res(ppA, ppB, gA, gB):
                    b2b = w["b2s"][:, :, None].to_broadcast((P, G, PT))
                    nc.vector.tensor_tensor(r[:, gA, :], ppA, xT_s[:, gA, ts], OP.add)
                    nc.vector.tensor_tensor(r[:, gB, :], ppB, xT_s[:, gB, ts], OP.add)
                    nc.vector.tensor_tensor(r[:, gA:gB + 1, :], r[:, gA:gB + 1, :],
                                            b2b[:, gA:gB + 1, :], OP.add)

                for gpair in range(2):
                    gA, gB = 2 * gpair, 2 * gpair + 1
                    ppA = pppool.tile([P, PT], F32, tag="pp", name=f"f2a_{li}_{p}_{gpair}")
                    ppB = pppool.tile([P, PT], F32, tag="pp", name=f"f2b_{li}_{p}_{gpair}")
                    for fi0 in (0, 4, 8, 12):
                        pieces.append(lambda ppA=ppA, ppB=ppB, gA=gA, gB=gB, fi0=fi0:
                                      mm(ppA, ppB, gA, gB, fi0))
                    pieces.append(lambda ppA=ppA, ppB=ppB, gA=gA, gB=gB:
                                  res(ppA, ppB, gA, gB))

                def ln2_out():
                    layernorm_pair(li, p, r, r2, w["g2s"], w["bt2s"], "b")
                    if li == total - 1:
                        # final layer: stream this pair's output now
                        for g in range(G):
                            nc.sync.dma_start(out_d[g * P:(g + 1) * P, ts],
                                              xT_s[:, g, ts])
                pieces.append(ln2_out)
                return pieces

            # ---------------- global pipeline ----------------
            steal = deque()
            setup(0)
            for piece in proj_pieces(0, 0):
                piece()
            for li in range(total):
                for p in range(NPAIR):
                    nxt = (li, p + 1) if p + 1 < NPAIR else \
                          ((li + 1, 0) if li + 1 < total else None)
                    if nxt is not None:
                        if nxt[1] == 0:
                            setup(nxt[0])
                        steal.extend(proj_pieces(*nxt))
                    prv = (li, p - 1) if p >= 1 else \
                          ((li - 1, NPAIR - 1) if li >= 1 else None)
                    if prv is not None:
                        steal.extend(ffn1_pieces(*prv))
                        steal.extend(ffn2_pieces(*prv))
                    attn(li, 2 * p, steal)
                    attn(li, 2 * p + 1, steal)
                    wo_ln1(li, p, steal)
            while steal:
                steal.popleft()()
            for piece in ffn1_pieces(total - 1, NPAIR - 1):
                piece()
            for piece in ffn2_pieces(total - 1, NPAIR - 1):
                piece()

    nc.compile()
    return nc


def _get_nc(n_layers=N_LAYERS, rep=1, abl=""):
    key = (n_layers, rep, abl)
    if key not in _CACHE:
        _CACHE[key] = _build(n_layers, rep, abl)
    return _CACHE[key]


def make_in_maps(x, wq, wk, wv, wo, w1, b1, w2, b2, g1, beta1, g2, beta2):
    import ml_dtypes
    BF = ml_dtypes.bfloat16
    x = np.asarray(x, np.float32)
    common = {
        "wq": np.ascontiguousarray(np.asarray(wq, np.float32).astype(BF)),
        "wk": np.ascontiguousarray(np.asarray(wk, np.float32).astype(BF)),
        "wv": np.ascontiguousarray(np.asarray(wv, np.float32).astype(BF)),
        "wo": np.ascontiguousarray(np.asarray(wo, np.float32).astype(BF)),
        "w1": np.ascontiguousarray(np.asarray(w1, np.float32).astype(BF)),
        "w2": np.ascontiguousarray(np.asarray(w2, np.float32).astype(BF)),
        "b1": np.ascontiguousarray(np.asarray(b1, np.float32)),
        "b2": np.ascontiguousarray(np.asarray(b2, np.float32)),
        "g1": np.ascontiguousarray(np.asarray(g1, np.float32)),
        "beta1": np.ascontiguousarray(np.asarray(beta1, np.float32)),
        "g2": np.ascontiguousarray(np.asarray(g2, np.float32)),
        "beta2": np.ascontiguousarray(np.asarray(beta2, np.float32)),
    }
    return [{"xT": np.ascontiguousarray(x[b].T.astype(BF)), **common}
            for b in range(B)]


def kernel(x, wq, wk, wv, wo, w1, b1, w2, b2, g1, beta1, g2, beta2,
           _n_layers=N_LAYERS, _trace=False):
    from concourse.bass_utils import run_bass_kernel_spmd

    nc = _get_nc(_n_layers)
    in_maps = make_in_maps(x, wq, wk, wv, wo, w1, b1, w2, b2,
                           g1, beta1, g2, beta2)
    res = run_bass_kernel_spmd(nc, in_maps, core_ids=list(range(B)), trace=_trace)
    out = np.stack([np.asarray(res.results[b]["out"]).astype(np.float32).T
                    for b in range(B)])
    if _trace:
        kernel.last_exec_time_ns = res.exec_time_ns
        kernel.last_results = res
    return out.astype(np.float32)

